# revision 2
# baseline (speedup 1.0000x reference)
"""Trainium2 Bass kernel for nn_GCNNLayer_56796647522692 (GCN message-passing layer).

Math (per flattened token row j of M = BNK*L = 25600, D = O = 1024, R = 50):
    idx      = adj_arc_in[:,0]*L + adj_arc_in[:,1]          (gather source rows)
    in_      = rep_[idx] @ W_in + b_in[lab]                 (gather commutes with matmul)
    in_gate  = rep_[idx] @ W_gate_in + b_gate_in[lab]
    same_    = rep_ @ W_self
    same_g   = rep_ @ W_gate_self
    w_in     = adj_mask_in^2  * sigmoid(in_gate)
    w_self   = adj_mask_loop^2 * sigmoid(same_g)
    out      = relu(in_*w_in + same_*w_self) * mask

Sharding: data-parallel over rows, 3200 rows/core on 8 cores. The host performs the
row gather (rep_[idx]), the lhsT-layout transposes, the label one-hot, and the mask
packing during input sharding; each core then runs a dense fused matmul kernel.

Device layout (token-major outputs, out partitions = tokens):
    lhsT = X^T 128x128 chunks (stationary), rhs = W 128x512 chunks (moving),
    PSUM accumulates over the 8 k-tiles; the b_in[lab] bias is added with one extra
    matmul lhsT=onehot(lab)^T (K=50).  Gates are N=1 matmuls sharing the same lhsT.
    Combine on ACT/DVE: sigmoid -> per-partition scales -> relu*mask -> DMA out.
"""

import numpy as np
import ml_dtypes

import concourse.bass as bass
import concourse.tile as tile
from concourse import bacc, mybir
from concourse.bass_utils import run_bass_kernel_spmd

# ---- problem dims (hardcoded per contract) ----
BNK, L, D, O, R = 200, 128, 1024, 1024, 50
M = BNK * L              # 25600
NCORES = 8
MC = M // NCORES         # 3200 rows per core
P = 128
MT = MC // P             # 25 m-tiles per core
KT = D // P              # 8 k-tiles
NFREE = 512
NT = O // NFREE          # 2 n-chunks

# matmul input dtype: "bf16" (1 cyc/row), "f32r" (1 cyc/row, reduced-precision fp32),
# "f32" (4 cyc/row, full fp32)
MM_MODE = "bf16"

_DT = {
    "bf16": (mybir.dt.bfloat16, ml_dtypes.bfloat16),
    "f32r": (mybir.dt.float32r, np.float32),
    "f32": (mybir.dt.float32, np.float32),
}
MM_DT, MM_NP = _DT[MM_MODE]
F32 = mybir.dt.float32
AF = mybir.ActivationFunctionType


def build_bass():
    nc = bacc.Bacc("TRN2", target_bir_lowering=False, debug=False, num_devices=NCORES)

    xt = nc.dram_tensor("xt", (MT, P, 2, KT, P), MM_DT, kind="ExternalInput").ap()
    w = nc.dram_tensor("w", (2, D, O), MM_DT, kind="ExternalInput").ap()
    wg = nc.dram_tensor("wg", (D, 2), MM_DT, kind="ExternalInput").ap()
    bau = nc.dram_tensor("bau", (R, O + 1), MM_DT, kind="ExternalInput").ap()
    oht = nc.dram_tensor("oht", (R, MC), MM_DT, kind="ExternalInput").ap()
    msk = nc.dram_tensor("msk", (P, MT, 3), F32, kind="ExternalInput").ap()
    out = nc.dram_tensor("out", (MC, O), F32, kind="ExternalOutput").ap()

    with tile.TileContext(nc) as tc:
        with (
            tc.tile_pool(name="const", bufs=1) as const,
            tc.tile_pool(name="xtp", bufs=4) as xtp,
            tc.tile_pool(name="colp", bufs=4) as colp,
            tc.tile_pool(name="tmp", bufs=4) as tmpp,
            tc.tile_pool(name="outp", bufs=4) as outp,
            tc.tile_pool(name="psum", bufs=6, space="PSUM") as psum,
            tc.tile_pool(name="psg", bufs=2, space="PSUM") as psg,
        ):
            # ---- static operands ----
            w_sb = const.tile([P, 2, KT, O], MM_DT)
            for s in range(2):
                nc.sync.dma_start(w_sb[:, s], w[s].rearrange("(k p) o -> p k o", p=P))
            wg_sb = const.tile([P, KT, 2], MM_DT)
            nc.sync.dma_start(wg_sb[:], wg.rearrange("(k p) g -> p k g", p=P))
            bau_sb = const.tile([R, O + 1], MM_DT)
            nc.sync.dma_start(bau_sb[:], bau)
            oht_sb = const.tile([R, MC], MM_DT)
            nc.sync.dma_start(oht_sb[:], oht)
            msk_sb = const.tile([P, MT, 3], F32)
            nc.sync.dma_start(msk_sb[:], msk)

            for m in range(MT):
                xt_t = xtp.tile([P, 2, KT, P], MM_DT)
                nc.sync.dma_start(xt_t[:], xt[m])

                i_ps = [psum.tile([P, NFREE], F32, tag="big", name=f"i_ps{n}")
                        for n in range(NT)]
                s_ps = [psum.tile([P, NFREE], F32, tag="big", name=f"s_ps{n}")
                        for n in range(NT)]
                g_ps = psg.tile([P, 2], F32)

                oh_m = oht_sb[:, m * P:(m + 1) * P]

                # in-arc: I[n] = Xin^T-chunks @ W_in chunks ; gin = Xin @ wg_in
                for k in range(KT):
                    lhsT = xt_t[:, 0, k]
                    for n in range(NT):
                        nc.tensor.matmul(
                            i_ps[n][:], lhsT, w_sb[:, 0, k, n * NFREE:(n + 1) * NFREE],
                            start=(k == 0), stop=False)
                    nc.tensor.matmul(
                        g_ps[:, 0:1], lhsT, wg_sb[:, k, 0:1],
                        start=(k == 0), stop=False)
                # + b_in[lab] and b_gate_in[lab] via one-hot matmul (K=50)
                for n in range(NT):
                    nc.tensor.matmul(
                        i_ps[n][:], oh_m, bau_sb[:, n * NFREE:(n + 1) * NFREE],
                        start=False, stop=True)
                nc.tensor.matmul(g_ps[:, 0:1], oh_m, bau_sb[:, O:O + 1],
                                 start=False, stop=True)

                # self-loop: S[n] = Xself @ W_self ; gs = Xself @ wg_self
                for k in range(KT):
                    lhsT = xt_t[:, 1, k]
                    for n in range(NT):
                        nc.tensor.matmul(
                            s_ps[n][:], lhsT, w_sb[:, 1, k, n * NFREE:(n + 1) * NFREE],
                            start=(k == 0), stop=(k == KT - 1))
                    nc.tensor.matmul(
                        g_ps[:, 1:2], lhsT, wg_sb[:, k, 1:2],
                        start=(k == 0), stop=(k == KT - 1))

                # gate weights: w = mask_soft^2 * sigmoid(gate)   (128,2)
                wcol = colp.tile([P, 2], F32)
                nc.scalar.activation(wcol[:], g_ps[:], AF.Sigmoid)
                nc.vector.tensor_tensor(wcol[:], wcol[:], msk_sb[:, m, 0:2],
                                        mybir.AluOpType.mult)

                # combine: out = relu(I*w_in + S*w_self) * mask
                for n in range(NT):
                    t1 = tmpp.tile([P, NFREE], F32, tag="t1")
                    t2 = tmpp.tile([P, NFREE], F32, tag="t2")
                    nc.scalar.mul(t1[:], i_ps[n][:], wcol[:, 0:1])
                    nc.vector.tensor_scalar_mul(t2[:], s_ps[n][:], wcol[:, 1:2])
                    nc.vector.tensor_add(t1[:], t1[:], t2[:])
                    o_t = outp.tile([P, NFREE], F32, tag="ot")
                    nc.scalar.activation(o_t[:], t1[:], AF.Relu,
                                         scale=msk_sb[:, m, 2:3])
                    nc.sync.dma_start(
                        out[m * P:(m + 1) * P, n * NFREE:(n + 1) * NFREE], o_t[:])

    nc.compile()
    return nc


_NC = None


def _get_nc():
    global _NC
    if _NC is None:
        _NC = build_bass()
    return _NC


def make_in_maps(rep, adj_arc_in, adj_lab_in, adj_mask_in, adj_mask_loop, mask,
                 W_in, b_in, W_gate_in, b_gate_in, W_self, W_gate_self):
    rep_ = np.ascontiguousarray(np.asarray(rep, dtype=np.float32)).reshape(M, D)
    arc = np.asarray(adj_arc_in)
    lab = np.asarray(adj_lab_in)
    idx = arc[:, 0].astype(np.int64) * L + arc[:, 1].astype(np.int64)
    gath = rep_[idx]  # (M, D)

    w_both = np.stack([np.asarray(W_in), np.asarray(W_self)]).astype(MM_NP)
    wg2 = np.concatenate([np.asarray(W_gate_in), np.asarray(W_gate_self)],
                         axis=1).astype(MM_NP)
    bau = np.concatenate([np.asarray(b_in), np.asarray(b_gate_in)],
                         axis=1).astype(MM_NP)

    m2i = (np.asarray(adj_mask_in)[:, 0].astype(np.float32)) ** 2
    m2l = (np.asarray(adj_mask_loop)[:, 0].astype(np.float32)) ** 2
    mk = np.asarray(mask, dtype=np.float32).reshape(M)

    in_maps = []
    for c in range(NCORES):
        rows = slice(c * MC, (c + 1) * MC)
        xb = np.stack([gath[rows], rep_[rows]])          # (2, MC, D) [s, j, d]
        v = xb.reshape(2, MT, P, KT, P)                  # [s, m, c, k, p]
        xt_c = np.ascontiguousarray(v.transpose(1, 4, 0, 3, 2)).astype(MM_NP)
        oht_c = (np.asarray(lab[rows])[None, :] == np.arange(R)[:, None]).astype(MM_NP)
        msk_c = np.ascontiguousarray(np.stack(
            [m2i[rows].reshape(MT, P).T,
             m2l[rows].reshape(MT, P).T,
             mk[rows].reshape(MT, P).T], axis=2)).astype(np.float32)
        in_maps.append({
            "xt": xt_c, "w": w_both, "wg": wg2, "bau": bau,
            "oht": oht_c, "msk": msk_c,
        })
    return in_maps


def kernel(**inputs):
    nc = _get_nc()
    in_maps = make_in_maps(**inputs)
    res = run_bass_kernel_spmd(nc, in_maps, core_ids=list(range(NCORES)))
    out = np.concatenate([res.results[c]["out"] for c in range(NCORES)], axis=0)
    return out.reshape(BNK, L, O)


# revision 23
# speedup vs baseline: 298.2212x; 298.2212x over previous
"""Trainium2 Bass kernel for nn_GCNNLayer_56796647522692 (GCN message-passing layer).

Math (per flattened token row j of M = BNK*L = 25600, D = O = 1024, R = 50):
    idx      = adj_arc_in[:,0]*L + adj_arc_in[:,1]          (gather source rows)
    in_      = rep_[idx] @ W_in + b_in[lab]                 (gather commutes with matmul)
    in_gate  = rep_[idx] @ W_gate_in + b_gate_in[lab]
    same_    = rep_ @ W_self
    same_g   = rep_ @ W_gate_self
    w_in     = adj_mask_in^2  * sigmoid(in_gate)
    w_self   = adj_mask_loop^2 * sigmoid(same_g)
    out      = relu(in_*w_in + same_*w_self) * mask

Sharding: data-parallel over rows, 3200 rows/core on 8 cores. The host performs the
row gather (rep_[idx]), the lhsT-layout transposes, the label one-hot, and the mask
packing during input sharding; each core then runs a dense fused matmul kernel.

Device layout (token-major outputs, out partitions = tokens):
    lhsT = X^T 128x128 chunks (stationary), rhs = W 128x512 chunks (moving),
    PSUM accumulates over the 8 k-tiles (fp16 inputs, fp32 accumulate; ~5e-4 rel
    err).  Gates ride the same lhsT as N=2 matmuls; b_gate_in[lab] comes from a
    one-hot matmul (K=50); b_in[lab] rows are host-gathered and added on VectorE.
    Combine on ACT/DVE: sigmoid -> per-partition scales -> relu*mask -> DMA out.
    Steady state is TensorE-saturated: 800 N=512 matmuls/core = 171us stream time,
    ~200us modeled end-to-end per core.
"""

import numpy as np
import ml_dtypes

import concourse.bass as bass
import concourse.tile as tile
from concourse import bacc, mybir
from concourse.bass_utils import run_bass_kernel_spmd

# ---- problem dims (hardcoded per contract) ----
BNK, L, D, O, R = 200, 128, 1024, 1024, 50
M = BNK * L              # 25600
NCORES = 8
MC = M // NCORES         # 3200 rows per core
P = 128
MT = MC // P             # 25 m-tiles per core
KT = D // P              # 8 k-tiles
NFREE = 512
NT = O // NFREE          # 2 n-chunks

# matmul input dtype: "bf16"/"f16" (1 cyc/row), "f32r" (1 cyc/row at N>=256,
# tf32-class precision), "f32" (4 cyc/row, full fp32)
import os
MM_MODE = os.environ.get("GCN_MM_MODE", "f16")
# bench-only: repeat the whole compute loop R times inside the NEFF so kernel
# time dominates the per-exec RPC overhead; slope between two R values gives HW time
REPEAT = int(os.environ.get("GCN_REPEAT", "1"))
# b_in[lab] add: "pe" = one-hot matmul on TensorE, "dve" = host-gathered rows
# added on VectorE (saves ~10us of PE time, costs extra DMA + DVE)
BIAS = os.environ.get("GCN_BIAS", "dve")
# timing probe only (wrong math): skip gate matmuls to measure their PE cost
NOGATE = os.environ.get("GCN_NOGATE", "0") == "1"
# PSUM slots: big pool bufs / gate pool bufs (8 banks total)
PSUM_BIG = int(os.environ.get("GCN_PSUM_BIG", "6"))
PSUM_G = int(os.environ.get("GCN_PSUM_G", "2"))
# per-m-tile emission order: "batch" = all I then all S, combine at end;
# "nphase" = per n-chunk {I_n, S_n, combine_n} so psum banks free earlier
ORDER = os.environ.get("GCN_ORDER", "nphase")

_DT = {
    "bf16": (mybir.dt.bfloat16, ml_dtypes.bfloat16),
    "f16": (mybir.dt.float16, np.float16),
    "f32r": (mybir.dt.float32r, np.float32),
    "f32": (mybir.dt.float32, np.float32),
}
MM_DT, MM_NP = _DT[MM_MODE]
F32 = mybir.dt.float32
AF = mybir.ActivationFunctionType


def build_bass():
    nc = bacc.Bacc("TRN2", target_bir_lowering=False, debug=False, num_devices=NCORES)

    xt = nc.dram_tensor("xt", (MT, P, 2, KT, P), MM_DT, kind="ExternalInput").ap()
    w = nc.dram_tensor("w", (2, D, O), MM_DT, kind="ExternalInput").ap()
    wg = nc.dram_tensor("wg", (D, 2), MM_DT, kind="ExternalInput").ap()
    bau = nc.dram_tensor("bau", (R, O + 2), MM_DT, kind="ExternalInput").ap()
    oht = nc.dram_tensor("oht", (R, MC), MM_DT, kind="ExternalInput").ap()
    msk = nc.dram_tensor("msk", (P, MT, 5), F32, kind="ExternalInput").ap()
    if BIAS == "dve":
        brow = nc.dram_tensor("brow", (MT, P, O), MM_DT, kind="ExternalInput").ap()
    out = nc.dram_tensor("out", (MC, O), F32, kind="ExternalOutput").ap()

    with tile.TileContext(nc) as tc:
        with (
            tc.tile_pool(name="const", bufs=1) as const,
            tc.tile_pool(name="xtp", bufs=4) as xtp,
            tc.tile_pool(name="colp", bufs=4) as colp,
            tc.tile_pool(name="tmp", bufs=4) as tmpp,
            tc.tile_pool(name="outp", bufs=4) as outp,
            tc.tile_pool(name="psum", bufs=PSUM_BIG, space="PSUM") as psum,
            tc.tile_pool(name="psg", bufs=PSUM_G, space="PSUM") as psg,
        ):
            # ---- static operands ----
            # per-(s,k) weight tiles so the first matmuls only wait on the first chunk
            w_t = [[const.tile([P, O], MM_DT, name=f"w_{s}_{k}") for k in range(KT)]
                   for s in range(2)]
            for k in range(KT):
                for s in range(2):
                    nc.sync.dma_start(w_t[s][k][:], w[s, k * P:(k + 1) * P, :])
            wg_sb = const.tile([P, KT, 2], MM_DT)
            nc.sync.dma_start(wg_sb[:], wg.rearrange("(k p) g -> p k g", p=P))
            bau_sb = const.tile([R, O + 2], MM_DT)
            nc.sync.dma_start(bau_sb[:], bau)
            oht_sb = const.tile([R, MC], MM_DT)
            nc.sync.dma_start(oht_sb[:], oht)
            msk_sb = const.tile([P, MT, 5], F32)
            nc.sync.dma_start(msk_sb[:], msk)

            for m in [mm for _ in range(REPEAT) for mm in range(MT)]:
                xt_t = xtp.tile([P, 2, KT, P], MM_DT)
                nc.sync.dma_start(xt_t[:], xt[m])
                if BIAS == "dve":
                    br_t = xtp.tile([P, O], MM_DT, tag="brow", name="br_t")
                    nc.sync.dma_start(br_t[:], brow[m])

                # gate psum: cols 0:2 = Xin @ [wg_in, wg_self], cols 2:4 = Xself @ same.
                # Only col 0 (g_in) and col 3 (g_self) are used; N=2 because f32r
                # matmuls reject a single-element free dim.
                g_ps = psg.tile([P, 4], F32)
                oh_m = oht_sb[:, m * P:(m + 1) * P]
                wcol = colp.tile([P, 4], F32)

                def mm_block(n, s, with_gates, m=m, xt_t=xt_t, g_ps=g_ps, oh_m=oh_m):
                    """8 k-tile matmuls of source s into a fresh psum tile for
                    n-chunk n; optionally ride the gate matmuls on the same lhsT."""
                    ps = psum.tile([P, NFREE], F32, tag="big", name=f"ps{s}{n}")
                    gsl = slice(0, 2) if s == 0 else slice(2, 4)
                    for k in range(KT):
                        lhsT = xt_t[:, s, k]
                        last = k == KT - 1
                        nc.tensor.matmul(
                            ps[:], lhsT, w_t[s][k][:, n * NFREE:(n + 1) * NFREE],
                            start=(k == 0),
                            stop=(last and (s == 1 or BIAS == "dve")))
                        if with_gates and not NOGATE:
                            nc.tensor.matmul(
                                g_ps[:, gsl], lhsT, wg_sb[:, k, 0:2],
                                start=(k == 0), stop=(last and s == 1))
                    if BIAS == "pe" and s == 0:
                        nc.tensor.matmul(
                            ps[:], oh_m, bau_sb[:, n * NFREE:(n + 1) * NFREE],
                            start=False, stop=True)
                    if with_gates and not NOGATE and s == 0:
                        # gate bias [b_gate_in[lab], 0] via one-hot
                        nc.tensor.matmul(g_ps[:, 0:2], oh_m, bau_sb[:, O:O + 2],
                                         start=False, stop=True)
                    return ps

                def finish_gates(m=m, g_ps=g_ps, wcol=wcol):
                    # gate weights: w = mask_soft^2 * sigmoid(gate); cols 0, 3 valid
                    if NOGATE:
                        nc.vector.tensor_copy(wcol[:], msk_sb[:, m, 0:4])
                    else:
                        nc.scalar.activation(wcol[:], g_ps[:], AF.Sigmoid)
                        nc.vector.tensor_tensor(wcol[:], wcol[:], msk_sb[:, m, 0:4],
                                                mybir.AluOpType.mult)

                def combine(n, ips, sps, m=m, wcol=wcol):
                    # out = relu((I+b)*w_in + S*w_self) * mask
                    t1 = tmpp.tile([P, NFREE], F32, tag="t1", name="t1")
                    t2 = tmpp.tile([P, NFREE], F32, tag="t2", name="t2")
                    if BIAS == "dve":
                        nc.vector.tensor_tensor(
                            t1[:], ips[:], br_t[:, n * NFREE:(n + 1) * NFREE],
                            mybir.AluOpType.add)
                        nc.scalar.mul(t1[:], t1[:], wcol[:, 0:1])
                    else:
                        nc.scalar.mul(t1[:], ips[:], wcol[:, 0:1])
                    nc.vector.tensor_scalar_mul(t2[:], sps[:], wcol[:, 3:4])
                    nc.vector.tensor_add(t1[:], t1[:], t2[:])
                    o_t = outp.tile([P, NFREE], F32, tag="ot", name="o_t")
                    nc.scalar.activation(o_t[:], t1[:], AF.Relu,
                                         scale=msk_sb[:, m, 4:5])
                    nc.sync.dma_start(
                        out[m * P:(m + 1) * P, n * NFREE:(n + 1) * NFREE], o_t[:])

                if ORDER == "batch":
                    i_ps = [mm_block(n, 0, with_gates=(n == 0)) for n in range(NT)]
                    s_ps = [mm_block(n, 1, with_gates=(n == 0)) for n in range(NT)]
                    finish_gates()
                    for n in range(NT):
                        combine(n, i_ps[n], s_ps[n])
                else:  # nphase: free each n-chunk's psum banks before the next
                    i0 = mm_block(0, 0, with_gates=True)
                    s0 = mm_block(0, 1, with_gates=True)
                    finish_gates()
                    combine(0, i0, s0)
                    i1 = mm_block(1, 0, with_gates=False)
                    s1 = mm_block(1, 1, with_gates=False)
                    combine(1, i1, s1)

    nc.compile()
    return nc


_NC = None


def _get_nc():
    global _NC
    if _NC is None:
        _NC = build_bass()
    return _NC


def make_in_maps(rep, adj_arc_in, adj_lab_in, adj_mask_in, adj_mask_loop, mask,
                 W_in, b_in, W_gate_in, b_gate_in, W_self, W_gate_self):
    rep_ = np.ascontiguousarray(np.asarray(rep, dtype=np.float32)).reshape(M, D)
    arc = np.asarray(adj_arc_in)
    lab = np.asarray(adj_lab_in)
    idx = arc[:, 0].astype(np.int64) * L + arc[:, 1].astype(np.int64)
    gath = rep_[idx]  # (M, D)

    w_both = np.stack([np.asarray(W_in), np.asarray(W_self)]).astype(MM_NP)
    wg2 = np.concatenate([np.asarray(W_gate_in), np.asarray(W_gate_self)],
                         axis=1).astype(MM_NP)
    bau = np.concatenate([np.asarray(b_in), np.asarray(b_gate_in),
                          np.zeros((R, 1), np.float32)], axis=1).astype(MM_NP)

    m2i = (np.asarray(adj_mask_in)[:, 0].astype(np.float32)) ** 2
    m2l = (np.asarray(adj_mask_loop)[:, 0].astype(np.float32)) ** 2
    mk = np.asarray(mask, dtype=np.float32).reshape(M)

    in_maps = []
    for c in range(NCORES):
        rows = slice(c * MC, (c + 1) * MC)
        xb = np.stack([gath[rows], rep_[rows]])          # (2, MC, D) [s, j, d]
        v = xb.reshape(2, MT, P, KT, P)                  # [s, m, c, k, p]
        xt_c = np.ascontiguousarray(v.transpose(1, 4, 0, 3, 2)).astype(MM_NP)
        oht_c = (np.asarray(lab[rows])[None, :] == np.arange(R)[:, None]).astype(MM_NP)
        zc = np.zeros((P, MT), np.float32)
        msk_c = np.ascontiguousarray(np.stack(
            [m2i[rows].reshape(MT, P).T, zc, zc,
             m2l[rows].reshape(MT, P).T,
             mk[rows].reshape(MT, P).T], axis=2)).astype(np.float32)
        im = {
            "xt": xt_c, "w": w_both, "wg": wg2, "bau": bau,
            "oht": oht_c, "msk": msk_c,
        }
        if BIAS == "dve":
            im["brow"] = np.asarray(b_in, dtype=np.float32)[
                lab[rows]].astype(MM_NP).reshape(MT, P, O)
        in_maps.append(im)
    return in_maps


def kernel(**inputs):
    nc = _get_nc()
    in_maps = make_in_maps(**inputs)
    res = run_bass_kernel_spmd(nc, in_maps, core_ids=list(range(NCORES)))
    out = np.concatenate([res.results[c]["out"] for c in range(NCORES)], axis=0)
    return out.reshape(BNK, L, O)


# revision 27
# speedup vs baseline: 303.0556x; 1.0162x over previous
"""Trainium2 Bass kernel for nn_GCNNLayer_56796647522692 (GCN message-passing layer).

Math (per flattened token row j of M = BNK*L = 25600, D = O = 1024, R = 50):
    idx      = adj_arc_in[:,0]*L + adj_arc_in[:,1]          (gather source rows)
    in_      = rep_[idx] @ W_in + b_in[lab]                 (gather commutes with matmul)
    in_gate  = rep_[idx] @ W_gate_in + b_gate_in[lab]
    same_    = rep_ @ W_self
    same_g   = rep_ @ W_gate_self
    w_in     = adj_mask_in^2  * sigmoid(in_gate)
    w_self   = adj_mask_loop^2 * sigmoid(same_g)
    out      = relu(in_*w_in + same_*w_self) * mask

Sharding: data-parallel over rows, 3200 rows/core on 8 cores. The host performs the
row gather (rep_[idx]), the lhsT-layout transposes, the label one-hot, and the mask
packing during input sharding; each core then runs a dense fused matmul kernel.

Device layout (token-major outputs, out partitions = tokens):
    lhsT = X^T 128x128 chunks (stationary), rhs = W 128x512 chunks (moving),
    PSUM accumulates over the 8 k-tiles (fp16 inputs, fp32 accumulate; ~5e-4 rel
    err).  Gates ride the same lhsT as N=2 matmuls; b_gate_in[lab] comes from a
    one-hot matmul (K=50); b_in[lab] rows are host-gathered and added on VectorE.
    Combine on ACT/DVE: sigmoid -> per-partition scales -> relu*mask -> DMA out.
    Steady state is TensorE-saturated: 800 N=512 matmuls/core = 171us stream time,
    ~200us modeled end-to-end per core.
"""

import numpy as np
import ml_dtypes

import concourse.bass as bass
import concourse.tile as tile
from concourse import bacc, mybir
from concourse.bass_utils import run_bass_kernel_spmd

# ---- problem dims (hardcoded per contract) ----
BNK, L, D, O, R = 200, 128, 1024, 1024, 50
M = BNK * L              # 25600
NCORES = 8
MC = M // NCORES         # 3200 rows per core
P = 128
MT = MC // P             # 25 m-tiles per core
KT = D // P              # 8 k-tiles
NFREE = 512
NT = O // NFREE          # 2 n-chunks

# matmul input dtype: "bf16"/"f16" (1 cyc/row), "f32r" (1 cyc/row at N>=256,
# tf32-class precision), "f32" (4 cyc/row, full fp32)
import os
MM_MODE = os.environ.get("GCN_MM_MODE", "f16")
# bench-only: repeat the whole compute loop R times inside the NEFF so kernel
# time dominates the per-exec RPC overhead; slope between two R values gives HW time
REPEAT = int(os.environ.get("GCN_REPEAT", "1"))
# b_in[lab] add: "pe" = one-hot matmul on TensorE, "dve" = host-gathered rows
# added on VectorE (saves ~10us of PE time, costs extra DMA + DVE)
BIAS = os.environ.get("GCN_BIAS", "dve")
# timing probe only (wrong math): skip gate matmuls to measure their PE cost
NOGATE = os.environ.get("GCN_NOGATE", "0") == "1"
# PSUM slots: big pool bufs / gate pool bufs (8 banks total)
PSUM_BIG = int(os.environ.get("GCN_PSUM_BIG", "6"))
PSUM_G = int(os.environ.get("GCN_PSUM_G", "2"))
# per-m-tile emission order: "batch" = all I then all S, combine at end;
# "nphase" = per n-chunk {I_n, S_n, combine_n} so psum banks free earlier
ORDER = os.environ.get("GCN_ORDER", "nphase")

_DT = {
    "bf16": (mybir.dt.bfloat16, ml_dtypes.bfloat16),
    "f16": (mybir.dt.float16, np.float16),
    "f32r": (mybir.dt.float32r, np.float32),
    "f32": (mybir.dt.float32, np.float32),
}
MM_DT, MM_NP = _DT[MM_MODE]
F32 = mybir.dt.float32
AF = mybir.ActivationFunctionType


def build_bass():
    nc = bacc.Bacc("TRN2", target_bir_lowering=False, debug=False, num_devices=NCORES)

    xt = nc.dram_tensor("xt", (MT, P, 2, KT, P), MM_DT, kind="ExternalInput").ap()
    w = nc.dram_tensor("w", (2, D, O), MM_DT, kind="ExternalInput").ap()
    wg = nc.dram_tensor("wg", (D, 2), MM_DT, kind="ExternalInput").ap()
    bau = nc.dram_tensor("bau", (R, O + 2), MM_DT, kind="ExternalInput").ap()
    oht = nc.dram_tensor("oht", (R, MC), MM_DT, kind="ExternalInput").ap()
    msk = nc.dram_tensor("msk", (P, MT, 5), F32, kind="ExternalInput").ap()
    if BIAS == "dve":
        brow = nc.dram_tensor("brow", (MT, P, O), MM_DT, kind="ExternalInput").ap()
    out = nc.dram_tensor("out", (MC, O), F32, kind="ExternalOutput").ap()

    with tile.TileContext(nc) as tc:
        with (
            tc.tile_pool(name="const", bufs=1) as const,
            tc.tile_pool(name="xtp", bufs=4) as xtp,
            tc.tile_pool(name="colp", bufs=4) as colp,
            tc.tile_pool(name="tmp", bufs=4) as tmpp,
            tc.tile_pool(name="outp", bufs=4) as outp,
            tc.tile_pool(name="psum", bufs=PSUM_BIG, space="PSUM") as psum,
            tc.tile_pool(name="psg", bufs=PSUM_G, space="PSUM") as psg,
        ):
            # first m-tile's inputs before the 4MB weight preload so the first
            # matmuls are not queued behind it
            xt0 = xtp.tile([P, 2, KT, P], MM_DT, tag="xt_t", name="xt0")
            nc.sync.dma_start(xt0[:], xt[0])
            br0 = None
            if BIAS == "dve":
                br0 = xtp.tile([P, O], MM_DT, tag="brow", name="br0")
                nc.sync.dma_start(br0[:], brow[0])

            # ---- static operands ----
            # per-(s,k) weight tiles so the first matmuls only wait on the first chunk
            w_t = [[const.tile([P, O], MM_DT, name=f"w_{s}_{k}") for k in range(KT)]
                   for s in range(2)]
            for k in range(KT):
                for s in range(2):
                    nc.sync.dma_start(w_t[s][k][:], w[s, k * P:(k + 1) * P, :])
            wg_sb = const.tile([P, KT, 2], MM_DT)
            nc.sync.dma_start(wg_sb[:], wg.rearrange("(k p) g -> p k g", p=P))
            bau_sb = const.tile([R, O + 2], MM_DT)
            nc.sync.dma_start(bau_sb[:], bau)
            oht_sb = const.tile([R, MC], MM_DT)
            nc.sync.dma_start(oht_sb[:], oht)
            msk_sb = const.tile([P, MT, 5], F32)
            nc.sync.dma_start(msk_sb[:], msk)

            first = True
            for m in [mm for _ in range(REPEAT) for mm in range(MT)]:
                if first and m == 0:
                    xt_t, br_t, first = xt0, br0, False
                else:
                    xt_t = xtp.tile([P, 2, KT, P], MM_DT, tag="xt_t", name="xt_t")
                    nc.sync.dma_start(xt_t[:], xt[m])
                    if BIAS == "dve":
                        br_t = xtp.tile([P, O], MM_DT, tag="brow", name="br_t")
                        nc.sync.dma_start(br_t[:], brow[m])

                # gate psum: cols 0:2 = Xin @ [wg_in, wg_self], cols 2:4 = Xself @ same.
                # Only col 0 (g_in) and col 3 (g_self) are used; N=2 because f32r
                # matmuls reject a single-element free dim.
                g_ps = psg.tile([P, 4], F32)
                oh_m = oht_sb[:, m * P:(m + 1) * P]
                wcol = colp.tile([P, 4], F32)

                def mm_block(n, s, with_gates, m=m, xt_t=xt_t, g_ps=g_ps, oh_m=oh_m):
                    """8 k-tile matmuls of source s into a fresh psum tile for
                    n-chunk n; optionally ride the gate matmuls on the same lhsT."""
                    ps = psum.tile([P, NFREE], F32, tag="big", name=f"ps{s}{n}")
                    gsl = slice(0, 2) if s == 0 else slice(2, 4)
                    for k in range(KT):
                        lhsT = xt_t[:, s, k]
                        last = k == KT - 1
                        nc.tensor.matmul(
                            ps[:], lhsT, w_t[s][k][:, n * NFREE:(n + 1) * NFREE],
                            start=(k == 0),
                            stop=(last and (s == 1 or BIAS == "dve")))
                        if with_gates and not NOGATE:
                            nc.tensor.matmul(
                                g_ps[:, gsl], lhsT, wg_sb[:, k, 0:2],
                                start=(k == 0), stop=(last and s == 1))
                    if BIAS == "pe" and s == 0:
                        nc.tensor.matmul(
                            ps[:], oh_m, bau_sb[:, n * NFREE:(n + 1) * NFREE],
                            start=False, stop=True)
                    if with_gates and not NOGATE and s == 0:
                        # gate bias [b_gate_in[lab], 0] via one-hot
                        nc.tensor.matmul(g_ps[:, 0:2], oh_m, bau_sb[:, O:O + 2],
                                         start=False, stop=True)
                    return ps

                def finish_gates(m=m, g_ps=g_ps, wcol=wcol):
                    # gate weights: w = mask_soft^2 * sigmoid(gate); cols 0, 3 valid
                    if NOGATE:
                        nc.vector.tensor_copy(wcol[:], msk_sb[:, m, 0:4])
                    else:
                        nc.scalar.activation(wcol[:], g_ps[:], AF.Sigmoid)
                        nc.vector.tensor_tensor(wcol[:], wcol[:], msk_sb[:, m, 0:4],
                                                mybir.AluOpType.mult)

                def combine(n, ips, sps, m=m, wcol=wcol):
                    # out = relu((I+b)*w_in + S*w_self) * mask
                    t1 = tmpp.tile([P, NFREE], F32, tag="t1", name="t1")
                    t2 = tmpp.tile([P, NFREE], F32, tag="t2", name="t2")
                    if BIAS == "dve":
                        nc.vector.tensor_tensor(
                            t1[:], ips[:], br_t[:, n * NFREE:(n + 1) * NFREE],
                            mybir.AluOpType.add)
                        nc.scalar.mul(t1[:], t1[:], wcol[:, 0:1])
                    else:
                        nc.scalar.mul(t1[:], ips[:], wcol[:, 0:1])
                    nc.vector.tensor_scalar_mul(t2[:], sps[:], wcol[:, 3:4])
                    nc.vector.tensor_add(t1[:], t1[:], t2[:])
                    o_t = outp.tile([P, NFREE], F32, tag="ot", name="o_t")
                    nc.scalar.activation(o_t[:], t1[:], AF.Relu,
                                         scale=msk_sb[:, m, 4:5])
                    nc.sync.dma_start(
                        out[m * P:(m + 1) * P, n * NFREE:(n + 1) * NFREE], o_t[:])

                if ORDER == "batch":
                    i_ps = [mm_block(n, 0, with_gates=(n == 0)) for n in range(NT)]
                    s_ps = [mm_block(n, 1, with_gates=(n == 0)) for n in range(NT)]
                    finish_gates()
                    for n in range(NT):
                        combine(n, i_ps[n], s_ps[n])
                else:  # nphase: free each n-chunk's psum banks before the next
                    i0 = mm_block(0, 0, with_gates=True)
                    s0 = mm_block(0, 1, with_gates=True)
                    finish_gates()
                    combine(0, i0, s0)
                    i1 = mm_block(1, 0, with_gates=False)
                    s1 = mm_block(1, 1, with_gates=False)
                    combine(1, i1, s1)

    nc.compile()
    return nc


_NC = None


def _get_nc():
    global _NC
    if _NC is None:
        _NC = build_bass()
    return _NC


def make_in_maps(rep, adj_arc_in, adj_lab_in, adj_mask_in, adj_mask_loop, mask,
                 W_in, b_in, W_gate_in, b_gate_in, W_self, W_gate_self):
    rep_ = np.ascontiguousarray(np.asarray(rep, dtype=np.float32)).reshape(M, D)
    arc = np.asarray(adj_arc_in)
    lab = np.asarray(adj_lab_in)
    idx = arc[:, 0].astype(np.int64) * L + arc[:, 1].astype(np.int64)
    gath = rep_[idx]  # (M, D)

    w_both = np.stack([np.asarray(W_in), np.asarray(W_self)]).astype(MM_NP)
    wg2 = np.concatenate([np.asarray(W_gate_in), np.asarray(W_gate_self)],
                         axis=1).astype(MM_NP)
    bau = np.concatenate([np.asarray(b_in), np.asarray(b_gate_in),
                          np.zeros((R, 1), np.float32)], axis=1).astype(MM_NP)

    m2i = (np.asarray(adj_mask_in)[:, 0].astype(np.float32)) ** 2
    m2l = (np.asarray(adj_mask_loop)[:, 0].astype(np.float32)) ** 2
    mk = np.asarray(mask, dtype=np.float32).reshape(M)

    in_maps = []
    for c in range(NCORES):
        rows = slice(c * MC, (c + 1) * MC)
        xb = np.stack([gath[rows], rep_[rows]])          # (2, MC, D) [s, j, d]
        v = xb.reshape(2, MT, P, KT, P)                  # [s, m, c, k, p]
        xt_c = np.ascontiguousarray(v.transpose(1, 4, 0, 3, 2)).astype(MM_NP)
        oht_c = (np.asarray(lab[rows])[None, :] == np.arange(R)[:, None]).astype(MM_NP)
        zc = np.zeros((P, MT), np.float32)
        msk_c = np.ascontiguousarray(np.stack(
            [m2i[rows].reshape(MT, P).T, zc, zc,
             m2l[rows].reshape(MT, P).T,
             mk[rows].reshape(MT, P).T], axis=2)).astype(np.float32)
        im = {
            "xt": xt_c, "w": w_both, "wg": wg2, "bau": bau,
            "oht": oht_c, "msk": msk_c,
        }
        if BIAS == "dve":
            im["brow"] = np.asarray(b_in, dtype=np.float32)[
                lab[rows]].astype(MM_NP).reshape(MT, P, O)
        in_maps.append(im)
    return in_maps


def kernel(**inputs):
    import time
    nc = _get_nc()
    in_maps = make_in_maps(**inputs)
    last = None
    for attempt in range(3):
        try:
            res = run_bass_kernel_spmd(nc, in_maps, core_ids=list(range(NCORES)))
            out = np.concatenate(
                [np.asarray(res.results[c]["out"]) for c in range(NCORES)], axis=0)
            return out.reshape(BNK, L, O)
        except Exception as e:  # transient device/tunnel errors: back off and retry
            last = e
            time.sleep(20 * (attempt + 1))
    raise last


# revision 30
# speedup vs baseline: 460.4159x; 1.5192x over previous
"""Trainium2 Bass kernel for nn_GCNNLayer_56796647522692 (GCN message-passing layer).

Math (per flattened token row j of M = BNK*L = 25600, D = O = 1024, R = 50):
    idx      = adj_arc_in[:,0]*L + adj_arc_in[:,1]          (gather source rows)
    in_      = rep_[idx] @ W_in + b_in[lab]                 (gather commutes with matmul)
    in_gate  = rep_[idx] @ W_gate_in + b_gate_in[lab]
    same_    = rep_ @ W_self
    same_g   = rep_ @ W_gate_self
    w_in     = adj_mask_in^2  * sigmoid(in_gate)
    w_self   = adj_mask_loop^2 * sigmoid(same_g)
    out      = relu(in_*w_in + same_*w_self) * mask

Sharding: data-parallel over rows, 3200 rows/core on 8 cores. The host performs the
row gather (rep_[idx]), the lhsT-layout transposes, the label one-hot, and the mask
packing during input sharding; each core then runs a dense fused matmul kernel.

Device layout (token-major outputs, out partitions = tokens):
    lhsT = X^T 128x128 chunks (stationary), rhs = W 128x512 chunks (moving),
    PSUM accumulates over the 8 k-tiles (fp16 inputs, fp32 accumulate; ~5e-4 rel
    err).  Gates ride the same lhsT as N=2 matmuls; b_gate_in[lab] is applied as
    the sigmoid's per-partition bias; b_in[lab] rows are host-gathered and added
    on VectorE.
    Combine on ACT/DVE: sigmoid -> per-partition scales -> relu*mask -> DMA out.
    Steady state is TensorE-saturated: 800 N=512 matmuls/core = 171us stream time,
    ~200us modeled end-to-end per core.
"""

import numpy as np
import ml_dtypes

import concourse.bass as bass
import concourse.tile as tile
from concourse import bacc, mybir
from concourse.bass_utils import run_bass_kernel_spmd

# ---- problem dims (hardcoded per contract) ----
BNK, L, D, O, R = 200, 128, 1024, 1024, 50
M = BNK * L              # 25600
NCORES = 8
MC = M // NCORES         # 3200 rows per core
P = 128
MT = MC // P             # 25 m-tiles per core
KT = D // P              # 8 k-tiles
NFREE = 512
NT = O // NFREE          # 2 n-chunks

# matmul input dtype: "bf16"/"f16" (1 cyc/row), "f32r" (1 cyc/row at N>=256,
# tf32-class precision), "f32" (4 cyc/row, full fp32)
import os
MM_MODE = os.environ.get("GCN_MM_MODE", "f16")
# bench-only: repeat the whole compute loop R times inside the NEFF so kernel
# time dominates the per-exec RPC overhead; slope between two R values gives HW time
REPEAT = int(os.environ.get("GCN_REPEAT", "1"))
# b_in[lab] add: "pe" = one-hot matmul on TensorE, "dve" = host-gathered rows
# added on VectorE (saves ~10us of PE time, costs extra DMA + DVE)
BIAS = os.environ.get("GCN_BIAS", "dve")
# timing probe only (wrong math): skip gate matmuls to measure their PE cost
NOGATE = os.environ.get("GCN_NOGATE", "0") == "1"
# PSUM slots: big pool bufs / gate pool bufs (8 banks total)
PSUM_BIG = int(os.environ.get("GCN_PSUM_BIG", "6"))
PSUM_G = int(os.environ.get("GCN_PSUM_G", "2"))
# per-m-tile emission order: "batch" = all I then all S, combine at end;
# "nphase" = per n-chunk {I_n, S_n, combine_n} so psum banks free earlier
ORDER = os.environ.get("GCN_ORDER", "nphase")

_DT = {
    "bf16": (mybir.dt.bfloat16, ml_dtypes.bfloat16),
    "f16": (mybir.dt.float16, np.float16),
    "f32r": (mybir.dt.float32r, np.float32),
    "f32": (mybir.dt.float32, np.float32),
}
MM_DT, MM_NP = _DT[MM_MODE]
F32 = mybir.dt.float32
AF = mybir.ActivationFunctionType


def build_bass():
    nc = bacc.Bacc("TRN2", target_bir_lowering=False, debug=False, num_devices=NCORES)

    xt = nc.dram_tensor("xt", (MT, P, 2, KT, P), MM_DT, kind="ExternalInput").ap()
    w = nc.dram_tensor("w", (2, D, O), MM_DT, kind="ExternalInput").ap()
    wg = nc.dram_tensor("wg", (D, 2), MM_DT, kind="ExternalInput").ap()
    if BIAS == "pe":
        bau = nc.dram_tensor("bau", (R, O), MM_DT, kind="ExternalInput").ap()
        oht = nc.dram_tensor("oht", (R, MC), MM_DT, kind="ExternalInput").ap()
    msk = nc.dram_tensor("msk", (P, MT, 6), F32, kind="ExternalInput").ap()
    if BIAS == "dve":
        brow = nc.dram_tensor("brow", (MT, P, O), MM_DT, kind="ExternalInput").ap()
    out = nc.dram_tensor("out", (MC, O), F32, kind="ExternalOutput").ap()

    with tile.TileContext(nc) as tc:
        with (
            tc.tile_pool(name="const", bufs=1) as const,
            tc.tile_pool(name="xtp", bufs=4) as xtp,
            tc.tile_pool(name="colp", bufs=4) as colp,
            tc.tile_pool(name="tmp", bufs=4) as tmpp,
            tc.tile_pool(name="outp", bufs=4) as outp,
            tc.tile_pool(name="psum", bufs=PSUM_BIG, space="PSUM") as psum,
            tc.tile_pool(name="psg", bufs=PSUM_G, space="PSUM") as psg,
        ):
            # first m-tile's inputs and the small constants before the 4MB weight
            # preload so the first matmuls are not queued behind it
            xt0 = xtp.tile([P, 2, KT, P], MM_DT, tag="xt_t", name="xt0")
            nc.sync.dma_start(xt0[:], xt[0])
            br0 = None
            if BIAS == "dve":
                br0 = xtp.tile([P, O], MM_DT, tag="brow", name="br0")
                nc.sync.dma_start(br0[:], brow[0])
            wg_sb = const.tile([P, KT, 2], MM_DT)
            nc.sync.dma_start(wg_sb[:], wg.rearrange("(k p) g -> p k g", p=P))
            if BIAS == "pe":
                bau_sb = const.tile([R, O], MM_DT)
                nc.sync.dma_start(bau_sb[:], bau)
                oht_sb = const.tile([R, MC], MM_DT)
                nc.sync.dma_start(oht_sb[:], oht)
            msk_sb = const.tile([P, MT, 6], F32)
            nc.sync.dma_start(msk_sb[:], msk)

            # ---- weight preload ----
            # per-(s,k) weight tiles so the first matmuls only wait on the first chunk
            w_t = [[const.tile([P, O], MM_DT, name=f"w_{s}_{k}") for k in range(KT)]
                   for s in range(2)]
            for k in range(KT):
                for s in range(2):
                    nc.sync.dma_start(w_t[s][k][:], w[s, k * P:(k + 1) * P, :])

            first = True
            for m in [mm for _ in range(REPEAT) for mm in range(MT)]:
                if first and m == 0:
                    xt_t, br_t, first = xt0, br0, False
                else:
                    xt_t = xtp.tile([P, 2, KT, P], MM_DT, tag="xt_t", name="xt_t")
                    nc.sync.dma_start(xt_t[:], xt[m])
                    if BIAS == "dve":
                        br_t = xtp.tile([P, O], MM_DT, tag="brow", name="br_t")
                        nc.sync.dma_start(br_t[:], brow[m])

                # gate psum: cols 0:2 = Xin @ [wg_in, wg_self], cols 2:4 = Xself @ same.
                # Only col 0 (g_in) and col 3 (g_self) are used; N=2 because f32r
                # matmuls reject a single-element free dim.
                g_ps = psg.tile([P, 4], F32)
                oh_m = oht_sb[:, m * P:(m + 1) * P] if BIAS == "pe" else None
                wcol = colp.tile([P, 4], F32)

                def mm_block(n, s, with_gates, m=m, xt_t=xt_t, g_ps=g_ps, oh_m=oh_m):
                    """8 k-tile matmuls of source s into a fresh psum tile for
                    n-chunk n; optionally ride the gate matmuls on the same lhsT."""
                    ps = psum.tile([P, NFREE], F32, tag="big", name=f"ps{s}{n}")
                    gsl = slice(0, 2) if s == 0 else slice(2, 4)
                    for k in range(KT):
                        lhsT = xt_t[:, s, k]
                        last = k == KT - 1
                        nc.tensor.matmul(
                            ps[:], lhsT, w_t[s][k][:, n * NFREE:(n + 1) * NFREE],
                            start=(k == 0),
                            stop=(last and (s == 1 or BIAS == "dve")))
                        if with_gates and not NOGATE:
                            nc.tensor.matmul(
                                g_ps[:, gsl], lhsT, wg_sb[:, k, 0:2],
                                start=(k == 0), stop=last)
                    if BIAS == "pe" and s == 0:
                        nc.tensor.matmul(
                            ps[:], oh_m, bau_sb[:, n * NFREE:(n + 1) * NFREE],
                            start=False, stop=True)
                    return ps

                def finish_gates(m=m, g_ps=g_ps, wcol=wcol):
                    # gate weights: w = mask_soft^2 * sigmoid(gate + gate_bias);
                    # cols 0, 3 valid.  b_gate_in[lab] rides msk col 5 and is applied
                    # as the sigmoid's per-partition bias (in-cols only).
                    if NOGATE:
                        nc.vector.tensor_copy(wcol[:], msk_sb[:, m, 0:4])
                    else:
                        nc.scalar.activation(wcol[:, 0:2], g_ps[:, 0:2], AF.Sigmoid,
                                             bias=msk_sb[:, m, 5:6])
                        nc.scalar.activation(wcol[:, 2:4], g_ps[:, 2:4], AF.Sigmoid)
                        nc.vector.tensor_tensor(wcol[:], wcol[:], msk_sb[:, m, 0:4],
                                                mybir.AluOpType.mult)

                def combine(n, ips, sps, m=m, wcol=wcol):
                    # out = relu((I+b)*w_in + S*w_self) * mask
                    t1 = tmpp.tile([P, NFREE], F32, tag="t1", name="t1")
                    t2 = tmpp.tile([P, NFREE], F32, tag="t2", name="t2")
                    if BIAS == "dve":
                        nc.vector.tensor_tensor(
                            t1[:], ips[:], br_t[:, n * NFREE:(n + 1) * NFREE],
                            mybir.AluOpType.add)
                        nc.scalar.mul(t1[:], t1[:], wcol[:, 0:1])
                    else:
                        nc.scalar.mul(t1[:], ips[:], wcol[:, 0:1])
                    nc.vector.tensor_scalar_mul(t2[:], sps[:], wcol[:, 3:4])
                    nc.vector.tensor_add(t1[:], t1[:], t2[:])
                    o_t = outp.tile([P, NFREE], F32, tag="ot", name="o_t")
                    nc.scalar.activation(o_t[:], t1[:], AF.Relu,
                                         scale=msk_sb[:, m, 4:5])
                    nc.sync.dma_start(
                        out[m * P:(m + 1) * P, n * NFREE:(n + 1) * NFREE], o_t[:])

                if ORDER == "batch":
                    i_ps = [mm_block(n, 0, with_gates=(n == 0)) for n in range(NT)]
                    s_ps = [mm_block(n, 1, with_gates=(n == 0)) for n in range(NT)]
                    finish_gates()
                    for n in range(NT):
                        combine(n, i_ps[n], s_ps[n])
                else:  # nphase: free each n-chunk's psum banks before the next
                    i0 = mm_block(0, 0, with_gates=True)
                    s0 = mm_block(0, 1, with_gates=True)
                    finish_gates()
                    combine(0, i0, s0)
                    i1 = mm_block(1, 0, with_gates=False)
                    s1 = mm_block(1, 1, with_gates=False)
                    combine(1, i1, s1)

    nc.compile()
    return nc


_NC = None


def _get_nc():
    global _NC
    if _NC is None:
        _NC = build_bass()
    return _NC


def make_in_maps(rep, adj_arc_in, adj_lab_in, adj_mask_in, adj_mask_loop, mask,
                 W_in, b_in, W_gate_in, b_gate_in, W_self, W_gate_self):
    rep_ = np.ascontiguousarray(np.asarray(rep, dtype=np.float32)).reshape(M, D)
    arc = np.asarray(adj_arc_in)
    lab = np.asarray(adj_lab_in)
    idx = arc[:, 0].astype(np.int64) * L + arc[:, 1].astype(np.int64)
    gath = rep_[idx]  # (M, D)

    w_both = np.stack([np.asarray(W_in), np.asarray(W_self)]).astype(MM_NP)
    wg2 = np.concatenate([np.asarray(W_gate_in), np.asarray(W_gate_self)],
                         axis=1).astype(MM_NP)
    bg = np.asarray(b_gate_in, dtype=np.float32)[:, 0]

    m2i = (np.asarray(adj_mask_in)[:, 0].astype(np.float32)) ** 2
    m2l = (np.asarray(adj_mask_loop)[:, 0].astype(np.float32)) ** 2
    mk = np.asarray(mask, dtype=np.float32).reshape(M)

    in_maps = []
    for c in range(NCORES):
        rows = slice(c * MC, (c + 1) * MC)
        xb = np.stack([gath[rows], rep_[rows]])          # (2, MC, D) [s, j, d]
        v = xb.reshape(2, MT, P, KT, P)                  # [s, m, c, k, p]
        xt_c = np.ascontiguousarray(v.transpose(1, 4, 0, 3, 2)).astype(MM_NP)
        zc = np.zeros((P, MT), np.float32)
        msk_c = np.ascontiguousarray(np.stack(
            [m2i[rows].reshape(MT, P).T, zc, zc,
             m2l[rows].reshape(MT, P).T,
             mk[rows].reshape(MT, P).T,
             bg[lab[rows]].reshape(MT, P).T], axis=2)).astype(np.float32)
        im = {"xt": xt_c, "w": w_both, "wg": wg2, "msk": msk_c}
        if BIAS == "pe":
            im["bau"] = np.asarray(b_in, dtype=np.float32).astype(MM_NP)
            im["oht"] = (np.asarray(lab[rows])[None, :] ==
                         np.arange(R)[:, None]).astype(MM_NP)
        if BIAS == "dve":
            im["brow"] = np.asarray(b_in, dtype=np.float32)[
                lab[rows]].astype(MM_NP).reshape(MT, P, O)
        in_maps.append(im)
    return in_maps


def kernel(**inputs):
    import time
    nc = _get_nc()
    in_maps = make_in_maps(**inputs)
    last = None
    for attempt in range(3):
        try:
            res = run_bass_kernel_spmd(nc, in_maps, core_ids=list(range(NCORES)))
            out = np.concatenate(
                [np.asarray(res.results[c]["out"]) for c in range(NCORES)], axis=0)
            return out.reshape(BNK, L, O)
        except Exception as e:  # transient device/tunnel errors: back off and retry
            last = e
            time.sleep(20 * (attempt + 1))
    raise last
